# revision 11
# baseline (speedup 1.0000x reference)
"""ErbEMA kernel for 8x TRN2 NeuronCores (Bass/Tile).

Problem: x [64,1,16384,32] f32, state [1,1,32];
  s_t = a*s_{t-1} + (1-a)*x_t ; out_t = (x_t - s_t)/40 ; a = 0.99.

Strategy (data-parallel over batch, 8 batches per core), per batch b:
  * Work in scaled domain s' = s/40, so out = 0.025*x - s'.
  * Load x as SBUF tile [128 part, 4096] where partition p holds
    t in [128p, 128p+128) (16KB contiguous/partition -> full-rate DMA).
  * ACT writes d1_f = c1*x in f-major layout [p, (f, t)] and
    xs = 0.025*x in t-major.
  * ONE DVE tensor_tensor_scan over the f-major tile gives the
    block-local recurrence s'_loc. The scan chains across lane
    boundaries (lane f inherits lane f-1's final); that contamination
    is linear and removed by the rank-1 correction below.
  * Correction needed at (p, t, f): add a^(t+1) * G[f,p] where
    G = carry-in (via lower-triangular B matmul over lane finals,
    plus the initial-state term) minus the lane-contamination term.
  * One PE matmul expands G to H'[p,(th,f)] = -G[f,p]*(a^16)^th
    (N=256, fp32); 16 DVE stt ops then apply
    out -= G*a^(t+1) as out += H' * a^(tl+1) over t = 16*th + tl,
    each op a contiguous-run [128, 256] slice — cheap.
  * GPSIMD computes out1 = xs - s'_loc (t-major view of f-major scan
    output) before the DVE correction; DMA out. Final state
    reconstructed from the last element + H'.
"""

import os

import numpy as np

B_FULL = 64
N_CORES = 8
B_LOC = B_FULL // N_CORES
T = 16384
F = 32
P = 128          # partitions = t-blocks per batch
TIN = T // P     # 128 time steps inside one partition
FREE = TIN * F   # 4096
TH = 8           # coarse decay steps: t = 16*th + tl
TL = 16
HN = TH * F      # 256

ALPHA = 0.99
A32 = np.float32(ALPHA)
C1 = np.float32((np.float64(1.0) - np.float64(A32)) / 40.0)   # (1-a)/40
OUT_SCALE = np.float32(0.025)
LAM = float(np.float64(ALPHA) ** TIN)                          # a^128
A16 = float(np.float64(ALPHA) ** TL)                           # a^16

_CACHE = {}


def _build_constants():
    # B[q, p] = lam^(p-1-q) for q <= p-1 else 0
    q = np.arange(P)[:, None].astype(np.float64)
    p = np.arange(P)[None, :].astype(np.float64)
    e = p - 1.0 - q
    bmat = np.where(e >= 0, np.float64(LAM) ** np.maximum(e, 0.0), 0.0)
    bmat = bmat.astype(np.float32)

    # D2[f', (th, f)] = delta(f',f) * (a^16)^th
    d2 = np.zeros((F, TH, F), dtype=np.float64)
    for f in range(F):
        d2[f, :, f] = np.float64(A16) ** np.arange(TH, dtype=np.float64)
    d2 = d2.reshape(F, HN).astype(np.float32)

    id128 = np.eye(P, dtype=np.float32)
    return bmat, d2, id128


def _build_program():
    import concourse.bacc as bacc
    import concourse.mybir as mybir
    from concourse import tile

    f32 = mybir.dt.float32
    nc = bacc.Bacc("TRN2", target_bir_lowering=False, debug=False,
                   num_devices=N_CORES)

    x_d = nc.dram_tensor("x", [B_LOC, T, F], f32, kind="ExternalInput")
    bm_d = nc.dram_tensor("bmat", [P, P], f32, kind="ExternalInput")
    d2_d = nc.dram_tensor("d2", [F, HN], f32, kind="ExternalInput")
    s0_d = nc.dram_tensor("s0c", [F, P], f32, kind="ExternalInput")
    id_d = nc.dram_tensor("id128", [P, P], f32, kind="ExternalInput")
    y_d = nc.dram_tensor("y", [B_LOC, T, F], f32, kind="ExternalOutput")
    fs_d = nc.dram_tensor("fstate", [B_LOC, F], f32, kind="ExternalOutput")

    xv = x_d.ap().rearrange("b (p q) f -> b p (q f)", p=P)
    yv = y_d.ap().rearrange("b (p q) f -> b p (q f)", p=P)

    mult = mybir.AluOpType.mult
    add = mybir.AluOpType.add

    with tile.TileContext(nc) as tc:
        with (
            tc.tile_pool(name="consts", bufs=1) as cp,
            tc.tile_pool(name="xin", bufs=3) as xp,
            tc.tile_pool(name="scan", bufs=2) as dp,
            tc.tile_pool(name="xsp", bufs=2) as sp,
            tc.tile_pool(name="outp", bufs=2) as op,
            tc.tile_pool(name="tiny", bufs=2) as tp,
            tc.tile_pool(name="cps", bufs=2, space="PSUM") as cpp,
            tc.tile_pool(name="hps", bufs=2, space="PSUM") as hpp,
        ):
            bt = cp.tile([P, P], f32)
            nc.sync.dma_start(bt[:], bm_d.ap())
            dt = cp.tile([F, HN], f32)
            nc.sync.dma_start(dt[:], d2_d.ap())
            st = cp.tile([F, P], f32)
            nc.sync.dma_start(st[:], s0_d.ap())
            idt = cp.tile([P, P], f32)
            nc.sync.dma_start(idt[:], id_d.ap())
            at = cp.tile([P, FREE], f32)
            nc.vector.memset(at[:], float(A32))

            for b in range(B_LOC):
                x_t = xp.tile([P, FREE], f32, tag="x")
                nc.sync.dma_start(x_t[:], xv[b])
                x_t3 = x_t[:].rearrange("p (t f) -> p t f", f=F)

                # d1_f = c1 * x, f-major (ACT transpose-write)
                d1 = dp.tile([P, FREE], f32, tag="d1")
                d1_w = d1[:].rearrange("p (f t) -> p t f", f=F)
                nc.scalar.mul(d1_w[:], x_t3[:], float(C1))

                # xs = 0.025 * x (t-major)
                xs = sp.tile([P, FREE], f32, tag="xs")
                nc.scalar.mul(xs[:], x_t[:], float(OUT_SCALE))

                # one in-place scan over the whole f-major tile
                nc.vector.tensor_tensor_scan(
                    d1[:], at[:], d1[:], initial=0.0, op0=mult, op1=add,
                )

                d1_3 = d1[:].rearrange("p (f t) -> p f t", f=F)

                # stage lane finals contiguously via ACT (strided DVE reads
                # stall badly under GPSIMD SBUF traffic)
                lc = tp.tile([P, F], f32, tag="lc")
                nc.scalar.copy(lc[:], d1_3[:, :, TIN - 1])

                # L_shift[:,0]=0, L_shift[:,f]=Lc[:,f-1]
                ls = tp.tile([P, F], f32, tag="ls")
                nc.vector.memset(ls[:, 0:1], 0.0)
                nc.vector.tensor_copy(ls[:, 1:F], lc[:, 0:F - 1])

                # L_true = Lc - lam * L_shift
                lt = tp.tile([P, F], f32, tag="lt")
                nc.vector.scalar_tensor_tensor(
                    out=lt[:], in0=ls[:], scalar=-LAM, in1=lc[:],
                    op0=mult, op1=add,
                )

                # carryT[f,p] = sum_q L_true[q,f] * B[q,p]
                cps = cpp.tile([F, P], f32, tag="carry")
                nc.tensor.matmul(cps[:], lt[:], bt[:], start=True, stop=True)

                # LshiftT[f,p] = L_shift[p,f]^T
                lst = cpp.tile([F, P], f32, tag="lsT")
                nc.tensor.transpose(lst[:], ls[:], idt[:])

                # G' = LshiftT - (carryT + S0c)   (negated correction)
                g2 = tp.tile([F, P], f32, tag="g2")
                nc.vector.tensor_add(g2[:], cps[:], st[:])
                gn = tp.tile([F, P], f32, tag="gn")
                nc.vector.tensor_sub(gn[:], lst[:], g2[:])

                # H'[p, (th, f)] = G'[f, p] * (a^16)^th  (one small matmul)
                hps = hpp.tile([P, HN], f32, tag="h")
                nc.tensor.matmul(hps[:], gn[:], dt[:], start=True, stop=True)

                # out1 = xs - s'_loc   (GPSIMD, t-major views)
                out1 = op.tile([P, FREE], f32, tag="o")
                o3 = out1[:].rearrange("p (t f) -> p t f", f=F)
                xs3 = xs[:].rearrange("p (t f) -> p t f", f=F)
                s3 = d1[:].rearrange("p (f t) -> p t f", f=F)
                nc.gpsimd.tensor_sub(o3[:], xs3[:], s3[:])

                # apply correction: out1 += H' * a^(tl+1) over t = 16*th+tl
                o4 = out1[:].rearrange("p (th tl f) -> p th tl f",
                                       th=TH, f=F)
                h3 = hps[:].rearrange("p (th f) -> p th f", f=F)
                for tl in range(TL):
                    nc.vector.scalar_tensor_tensor(
                        out=o4[:, :, tl, :], in0=h3[:],
                        scalar=float(np.float64(ALPHA) ** (tl + 1)),
                        in1=o4[:, :, tl, :], op0=mult, op1=add,
                    )

                nc.sync.dma_start(yv[b], out1[:])

                # final state = 40*(s'_loc[last] - H'[last]*a^16)
                fs_t = tp.tile([P, F], f32, tag="fs")
                nc.vector.scalar_tensor_tensor(
                    out=fs_t[64:P, :], in0=hps[64:P, HN - F:],
                    scalar=-float(A16), in1=lc[64:P, :],
                    op0=mult, op1=add,
                )
                nc.vector.tensor_scalar_mul(
                    fs_t[64:P, :], fs_t[64:P, :], 40.0,
                )
                nc.sync.dma_start(fs_d.ap()[b:b + 1, :], fs_t[P - 1:P, :])

    nc.compile()
    return nc


def _get_program():
    if "nc" not in _CACHE:
        _CACHE["nc"] = _build_program()
        _CACHE["consts"] = _build_constants()
    return _CACHE["nc"], _CACHE["consts"]


def kernel(feat_erb: np.ndarray, state: np.ndarray):
    from concourse.bass_utils import run_bass_kernel_spmd

    nc, (bmat, d2, id128) = _get_program()

    x = np.ascontiguousarray(feat_erb.reshape(B_FULL, T, F), dtype=np.float32)
    s0 = np.asarray(state, dtype=np.float32).reshape(F)

    # S0c[f, p] = 0.025 * s0[f] * lam^p
    lam_p = np.float64(LAM) ** np.arange(P, dtype=np.float64)
    s0c = (0.025 * s0.astype(np.float64)[:, None] * lam_p[None, :]).astype(
        np.float32)

    in_maps = []
    for i in range(N_CORES):
        in_maps.append({
            "x": x[i * B_LOC:(i + 1) * B_LOC],
            "bmat": bmat,
            "d2": d2,
            "s0c": s0c,
            "id128": id128,
        })

    trace = bool(int(os.environ.get("KERNEL_TRACE", "0")))
    res = run_bass_kernel_spmd(nc, in_maps, list(range(N_CORES)),
                               trace=trace)
    _CACHE["last_result"] = res

    y = np.concatenate([r["y"] for r in res.results], axis=0)
    fs = np.concatenate([r["fstate"] for r in res.results], axis=0)

    feat_out = y.reshape(B_FULL, 1, T, F)
    final_state = fs.reshape(B_FULL, 1, F)
    return feat_out, final_state


# revision 13
# speedup vs baseline: 1.0084x; 1.0084x over previous
"""ErbEMA kernel for 8x TRN2 NeuronCores (Bass/Tile).

Problem: x [64,1,16384,32] f32, state [1,1,32];
  s_t = a*s_{t-1} + (1-a)*x_t ; out_t = (x_t - s_t)/40 ; a = 0.99.

Strategy (data-parallel over batch, 8 batches per core), per batch b:
  * Work in scaled domain s' = s/40, so out = 0.025*x - s'.
  * Load x as SBUF tile [128 part, 4096] where partition p holds
    t in [128p, 128p+128) (16KB contiguous/partition -> full-rate DMA).
  * ACT writes d1_f = c1*x in f-major layout [p, (f, t)] and
    xs = 0.025*x in t-major.
  * ONE DVE tensor_tensor_scan over the f-major tile gives the
    block-local recurrence s'_loc. The scan chains across lane
    boundaries (lane f inherits lane f-1's final); that contamination
    is linear and removed by the rank-1 correction below.
  * Correction needed at (p, t, f): add a^(t+1) * G[f,p] where
    G = carry-in (via lower-triangular B matmul over lane finals,
    plus the initial-state term) minus the lane-contamination term.
  * One PE matmul expands G to H'[p,(th,f)] = -G[f,p]*(a^8)^th
    (N=512, fp32); 8 DVE stt ops then apply
    out -= G*a^(t+1) as out += H' * a^(tl+1) over t = 8*th + tl,
    each op a contiguous-run [128, 512] slice — cheap.
  * GPSIMD computes out1 = xs - s'_loc (t-major view of f-major scan
    output) before the DVE correction; DMA out. Final state
    reconstructed from the last element + H'.
"""

import os

import numpy as np

B_FULL = 64
N_CORES = 8
B_LOC = B_FULL // N_CORES
T = 16384
F = 32
P = 128          # partitions = t-blocks per batch
TIN = T // P     # 128 time steps inside one partition
FREE = TIN * F   # 4096
TH = 16          # coarse decay steps: t = 8*th + tl
TL = 8
HN = TH * F      # 256

ALPHA = 0.99
A32 = np.float32(ALPHA)
C1 = np.float32((np.float64(1.0) - np.float64(A32)) / 40.0)   # (1-a)/40
OUT_SCALE = np.float32(0.025)
LAM = float(np.float64(ALPHA) ** TIN)                          # a^128
A16 = float(np.float64(ALPHA) ** TL)                           # a^8

_CACHE = {}


def _build_constants():
    # B[q, p] = lam^(p-1-q) for q <= p-1 else 0
    q = np.arange(P)[:, None].astype(np.float64)
    p = np.arange(P)[None, :].astype(np.float64)
    e = p - 1.0 - q
    bmat = np.where(e >= 0, np.float64(LAM) ** np.maximum(e, 0.0), 0.0)
    bmat = bmat.astype(np.float32)

    # D2[f', (th, f)] = delta(f',f) * (a^16)^th
    d2 = np.zeros((F, TH, F), dtype=np.float64)
    for f in range(F):
        d2[f, :, f] = np.float64(A16) ** np.arange(TH, dtype=np.float64)
    d2 = d2.reshape(F, HN).astype(np.float32)

    id128 = np.eye(P, dtype=np.float32)
    return bmat, d2, id128


def _build_program():
    import concourse.bacc as bacc
    import concourse.mybir as mybir
    from concourse import tile

    f32 = mybir.dt.float32
    nc = bacc.Bacc("TRN2", target_bir_lowering=False, debug=False,
                   num_devices=N_CORES)

    x_d = nc.dram_tensor("x", [B_LOC, T, F], f32, kind="ExternalInput")
    bm_d = nc.dram_tensor("bmat", [P, P], f32, kind="ExternalInput")
    d2_d = nc.dram_tensor("d2", [F, HN], f32, kind="ExternalInput")
    s0_d = nc.dram_tensor("s0c", [F, P], f32, kind="ExternalInput")
    id_d = nc.dram_tensor("id128", [P, P], f32, kind="ExternalInput")
    y_d = nc.dram_tensor("y", [B_LOC, T, F], f32, kind="ExternalOutput")
    fs_d = nc.dram_tensor("fstate", [B_LOC, F], f32, kind="ExternalOutput")

    xv = x_d.ap().rearrange("b (p q) f -> b p (q f)", p=P)
    yv = y_d.ap().rearrange("b (p q) f -> b p (q f)", p=P)

    mult = mybir.AluOpType.mult
    add = mybir.AluOpType.add

    with tile.TileContext(nc) as tc:
        with (
            tc.tile_pool(name="consts", bufs=1) as cp,
            tc.tile_pool(name="xin", bufs=3) as xp,
            tc.tile_pool(name="scan", bufs=2) as dp,
            tc.tile_pool(name="xsp", bufs=2) as sp,
            tc.tile_pool(name="outp", bufs=2) as op,
            tc.tile_pool(name="tiny", bufs=2) as tp,
            tc.tile_pool(name="cps", bufs=2, space="PSUM") as cpp,
            tc.tile_pool(name="hps", bufs=2, space="PSUM") as hpp,
        ):
            bt = cp.tile([P, P], f32)
            nc.sync.dma_start(bt[:], bm_d.ap())
            dt = cp.tile([F, HN], f32)
            nc.sync.dma_start(dt[:], d2_d.ap())
            st = cp.tile([F, P], f32)
            nc.sync.dma_start(st[:], s0_d.ap())
            idt = cp.tile([P, P], f32)
            nc.sync.dma_start(idt[:], id_d.ap())
            at = cp.tile([P, FREE], f32)
            nc.vector.memset(at[:], float(A32))

            for b in range(B_LOC):
                x_t = xp.tile([P, FREE], f32, tag="x")
                nc.sync.dma_start(x_t[:], xv[b])
                x_t3 = x_t[:].rearrange("p (t f) -> p t f", f=F)

                # d1_f = c1 * x, f-major (ACT transpose-write)
                d1 = dp.tile([P, FREE], f32, tag="d1")
                d1_w = d1[:].rearrange("p (f t) -> p t f", f=F)
                nc.scalar.mul(d1_w[:], x_t3[:], float(C1))

                # xs = 0.025 * x (t-major)
                xs = sp.tile([P, FREE], f32, tag="xs")
                nc.scalar.mul(xs[:], x_t[:], float(OUT_SCALE))

                # one in-place scan over the whole f-major tile
                nc.vector.tensor_tensor_scan(
                    d1[:], at[:], d1[:], initial=0.0, op0=mult, op1=add,
                )

                d1_3 = d1[:].rearrange("p (f t) -> p f t", f=F)

                # stage lane finals contiguously via ACT (strided DVE reads
                # stall badly under GPSIMD SBUF traffic)
                lc = tp.tile([P, F], f32, tag="lc")
                nc.scalar.copy(lc[:], d1_3[:, :, TIN - 1])

                # L_shift[:,0]=0, L_shift[:,f]=Lc[:,f-1]
                ls = tp.tile([P, F], f32, tag="ls")
                nc.scalar.mul(ls[:, 0:1], lc[:, 0:1], 0.0)
                nc.scalar.copy(ls[:, 1:F], lc[:, 0:F - 1])

                # L_true = Lc - lam * L_shift
                lt = tp.tile([P, F], f32, tag="lt")
                nc.vector.scalar_tensor_tensor(
                    out=lt[:], in0=ls[:], scalar=-LAM, in1=lc[:],
                    op0=mult, op1=add,
                )

                # carryT[f,p] = sum_q L_true[q,f] * B[q,p]
                cps = cpp.tile([F, P], f32, tag="carry")
                nc.tensor.matmul(cps[:], lt[:], bt[:], start=True, stop=True)

                # LshiftT[f,p] = L_shift[p,f]^T
                lst = cpp.tile([F, P], f32, tag="lsT")
                nc.tensor.transpose(lst[:], ls[:], idt[:])

                # G' = LshiftT - (carryT + S0c)   (negated correction)
                g2 = tp.tile([F, P], f32, tag="g2")
                nc.vector.tensor_add(g2[:], cps[:], st[:])
                gn = tp.tile([F, P], f32, tag="gn")
                nc.vector.tensor_sub(gn[:], lst[:], g2[:])

                # H'[p, (th, f)] = G'[f, p] * (a^16)^th  (one small matmul)
                hps = hpp.tile([P, HN], f32, tag="h")
                nc.tensor.matmul(hps[:], gn[:], dt[:], start=True, stop=True)

                # out1 = xs - s'_loc   (GPSIMD, t-major views)
                out1 = op.tile([P, FREE], f32, tag="o")
                o3 = out1[:].rearrange("p (t f) -> p t f", f=F)
                xs3 = xs[:].rearrange("p (t f) -> p t f", f=F)
                s3 = d1[:].rearrange("p (f t) -> p t f", f=F)
                nc.gpsimd.tensor_sub(o3[:], xs3[:], s3[:])

                # apply correction: out1 += H' * a^(tl+1) over t = 16*th+tl
                o4 = out1[:].rearrange("p (th tl f) -> p th tl f",
                                       th=TH, f=F)
                h3 = hps[:].rearrange("p (th f) -> p th f", f=F)
                for tl in range(TL):
                    nc.vector.scalar_tensor_tensor(
                        out=o4[:, :, tl, :], in0=h3[:],
                        scalar=float(np.float64(ALPHA) ** (tl + 1)),
                        in1=o4[:, :, tl, :], op0=mult, op1=add,
                    )

                nc.sync.dma_start(yv[b], out1[:])

                # final state = 40*(s'_loc[last] - H'[last]*a^16)
                fs_t = tp.tile([P, F], f32, tag="fs")
                nc.vector.scalar_tensor_tensor(
                    out=fs_t[64:P, :], in0=hps[64:P, HN - F:],
                    scalar=-float(A16), in1=lc[64:P, :],
                    op0=mult, op1=add,
                )
                nc.vector.tensor_scalar_mul(
                    fs_t[64:P, :], fs_t[64:P, :], 40.0,
                )
                nc.sync.dma_start(fs_d.ap()[b:b + 1, :], fs_t[P - 1:P, :])

    nc.compile()
    return nc


def _get_program():
    if "nc" not in _CACHE:
        _CACHE["nc"] = _build_program()
        _CACHE["consts"] = _build_constants()
    return _CACHE["nc"], _CACHE["consts"]


def kernel(feat_erb: np.ndarray, state: np.ndarray):
    from concourse.bass_utils import run_bass_kernel_spmd

    nc, (bmat, d2, id128) = _get_program()

    x = np.ascontiguousarray(feat_erb.reshape(B_FULL, T, F), dtype=np.float32)
    s0 = np.asarray(state, dtype=np.float32).reshape(F)

    # S0c[f, p] = 0.025 * s0[f] * lam^p
    lam_p = np.float64(LAM) ** np.arange(P, dtype=np.float64)
    s0c = (0.025 * s0.astype(np.float64)[:, None] * lam_p[None, :]).astype(
        np.float32)

    in_maps = []
    for i in range(N_CORES):
        in_maps.append({
            "x": x[i * B_LOC:(i + 1) * B_LOC],
            "bmat": bmat,
            "d2": d2,
            "s0c": s0c,
            "id128": id128,
        })

    trace = bool(int(os.environ.get("KERNEL_TRACE", "0")))
    res = run_bass_kernel_spmd(nc, in_maps, list(range(N_CORES)),
                               trace=trace)
    _CACHE["last_result"] = res

    y = np.concatenate([r["y"] for r in res.results], axis=0)
    fs = np.concatenate([r["fstate"] for r in res.results], axis=0)

    feat_out = y.reshape(B_FULL, 1, T, F)
    final_state = fs.reshape(B_FULL, 1, F)
    return feat_out, final_state
